# revision 1
# baseline (speedup 1.0000x reference)
"""Trainium2 Bass kernel for the Cross_AttentionBlock problem.

Data-parallel over batch: B=8 samples -> 8 NeuronCores, one sample per core.
All compute for a sample runs on its core; no collectives.

Math restructure vs the reference (all exact-math-equivalent):
  - vis = wv_w @ key is never materialized; attention logits use
    associativity:  logits = (query @ wv_w) @ key + query @ wv_b.
  - softmax(axis=S) -> mask -> renormalize == masked softmax; implemented as
    exp(logit - C1 + mask_bias) / colsum with a fixed shift C1=64
    (safe: |logits| <= ~125, exp argument stays well inside fp32 range).
  - second softmax needs no shift (|sim * KC^-0.5| <= ~7).
  - fv_b folds into ctx via attn columns summing to 1 *before* normalize:
    ctx_unnorm += fv_b (x) colsum  (one K=1 matmul per c-chunk), then
    ctx = ctx_unnorm * recip(colsum).
  - out2 = w2 @ ctx + (w1 @ [ctx; wemb]) accumulates in one PSUM group with
    combined bias bo = w1_b + w2_b.

Matmuls run as float32r (full PE rate at free-dim >= 256) except the
word-attention logit path, which is plain fp32 because those logits are
O(100) and feed exp() -- absolute accuracy matters there.
"""

import sys

for _p in ("/opt/pypackages", "/opt/trn_rl_repo"):
    if _p not in sys.path:
        sys.path.insert(0, _p)

import numpy as np

import concourse.bass as bass
import concourse.tile as tile
from concourse import bacc, mybir

P = 128
B = 8
S = 20
C = 512
HW = 48 * 48  # 2304
KC = 256
H = W = 48
C1 = 64.0  # fixed softmax shift for the word-attention logits
MASK_NEG = -1e30
SCALE2 = KC ** -0.5

F32 = mybir.dt.float32
F32R = mybir.dt.float32r
AF = mybir.ActivationFunctionType
OP = mybir.AluOpType

# n-tiles over the 2304 spatial positions
N_TILES = [(0, 512), (512, 512), (1024, 512), (1536, 512), (2048, 256)]
M_CHUNKS = HW // P  # 18


def _declare_io(nc):
    din = {}  # DRAM input APs
    # float32r tensors feed full-rate PE matmuls; walrus requires f32r
    # consumers to read f32r-produced data, so they are typed end-to-end.
    for name, shape, dt_ in [
        ("key", (C, HW), F32),
        ("key_r", (C, HW), F32R),
        ("query", (S, C), F32R),
        ("query_t", (C, S), F32),
        ("abias", (S, 1), F32),
        ("wv_wF", (C, C), F32),
        ("fk_wT", (C, KC), F32R),
        ("fq_wT", (C, KC), F32R),
        ("fv_wT", (C, C), F32R),
        ("w1aT", (C, C), F32R),
        ("w1bT", (C, C), F32R),
        ("w2T", (C, C), F32R),
        ("wv_b", (C, 1), F32),
        ("fk_b", (KC, 1), F32),
        ("fq_b", (KC, 1), F32),
        ("fv_b_row", (1, C), F32R),
        ("bo", (C, 1), F32),
        ("ones_col", (P, 1), F32R),
    ]:
        din[name] = nc.dram_tensor(name, list(shape), dt_, kind="ExternalInput").ap()
    out_d = nc.dram_tensor("out", [C, HW], F32, kind="ExternalOutput").ap()
    return din, out_d


def _build_body(nc, tc, din, out_d, rep=0):
    key_v = din["key"].rearrange("(c p) n -> p c n", p=P)      # [128, 4, 2304]
    key_rv = din["key_r"].rearrange("(c p) n -> p c n", p=P)   # f32r copy
    out_v = out_d.rearrange("(c p) n -> p c n", p=P)           # [128, 4, 2304]

    wpool = tc.tile_pool(name=f"wpool{rep}", bufs=1)
    apool = tc.tile_pool(name=f"apool{rep}", bufs=1)
    with wpool as wp, apool as ap_:
        # ---- persistent weights / small constants ----
        fk_wT = wp.tile([P, 4, KC], F32R, tag="fk_wT")
        nc.sync.dma_start(fk_wT, din["fk_wT"].rearrange("(c p) o -> p c o", p=P))
        fq_wT = wp.tile([P, 4, KC], F32R, tag="fq_wT")
        nc.sync.dma_start(fq_wT, din["fq_wT"].rearrange("(c p) o -> p c o", p=P))
        fv_wT = wp.tile([P, 4, C], F32R, tag="fv_wT")
        nc.sync.dma_start(fv_wT, din["fv_wT"].rearrange("(c p) o -> p c o", p=P))
        w1aT = wp.tile([P, 4, C], F32R, tag="w1aT")
        nc.sync.dma_start(w1aT, din["w1aT"].rearrange("(c p) o -> p c o", p=P))
        w1bT = wp.tile([P, 4, C], F32R, tag="w1bT")
        nc.sync.dma_start(w1bT, din["w1bT"].rearrange("(c p) o -> p c o", p=P))
        w2T = wp.tile([P, 4, C], F32R, tag="w2T")
        nc.sync.dma_start(w2T, din["w2T"].rearrange("(c p) o -> p c o", p=P))

        query_t = wp.tile([P, 4, S], F32, tag="query_t")
        nc.sync.dma_start(query_t, din["query_t"].rearrange("(c p) s -> p c s", p=P))
        query = wp.tile([S, C], F32R, tag="query")
        nc.sync.dma_start(query, din["query"])
        abias = wp.tile([S, 1], F32, tag="abias")
        nc.sync.dma_start(abias, din["abias"])

        fk_b = wp.tile([P, 2, 1], F32, tag="fk_b")
        nc.sync.dma_start(fk_b, din["fk_b"].rearrange("(c p) o -> p c o", p=P))
        fq_b = wp.tile([P, 2, 1], F32, tag="fq_b")
        nc.sync.dma_start(fq_b, din["fq_b"].rearrange("(c p) o -> p c o", p=P))
        wv_b = wp.tile([P, 4, 1], F32, tag="wv_b")
        nc.sync.dma_start(wv_b, din["wv_b"].rearrange("(c p) o -> p c o", p=P))
        bo = wp.tile([P, 4, 1], F32, tag="bo")
        nc.sync.dma_start(bo, din["bo"].rearrange("(c p) o -> p c o", p=P))
        fv_b_row = wp.tile([1, C], F32R, tag="fv_b_row")
        nc.sync.dma_start(fv_b_row, din["fv_b_row"])

        ones_col = wp.tile([P, 1], F32R, tag="ones_col")
        nc.sync.dma_start(ones_col, din["ones_col"])
        ones128 = ones_col
        ones20 = ones_col[:S, :]
        ones_1x128 = wp.tile([1, P], F32, tag="ones_1x128")
        nc.vector.memset(ones_1x128, 1.0)
        ones_1x20 = wp.tile([1, S], F32, tag="ones_1x20")
        nc.vector.memset(ones_1x20, 1.0)

        # ---- long-lived activations ----
        wemb = ap_.tile([P, 4, HW], F32R, tag="wemb")
        kf = ap_.tile([P, 2, HW], F32R, tag="kf")
        qf = ap_.tile([P, 2, HW], F32R, tag="qf")
        vt = ap_.tile([P, M_CHUNKS, C], F32R, tag="vt")

        # ================= phase 1 =================
        with tc.tile_pool(name=f"p1sb{rep}", bufs=1) as p1, \
             tc.tile_pool(name=f"p1key{rep}", bufs=2) as p1k, \
             tc.tile_pool(name=f"p1tmp{rep}", bufs=1) as p1t, \
             tc.tile_pool(name=f"p1cp{rep}", bufs=4) as p1cp, \
             tc.tile_pool(name=f"p1ps{rep}", bufs=2, space="PSUM") as pp1, \
             tc.tile_pool(name=f"p1psx{rep}", bufs=1, space="PSUM") as pp1x, \
             tc.tile_pool(name=f"p1pscv{rep}", bufs=3, space="PSUM") as ppcv:
            # wv_w UNtransposed: qv contracts over wv_w's FIRST (output) index
            wv_wF = p1.tile([P, 4, C], F32, tag="wv_wF")
            nc.sync.dma_start(wv_wF, din["wv_wF"].rearrange("(o p) c -> p o c", p=P))

            # qv_T[c', s] = sum_o wv_w[o, c'] query[s, o]  (fp32, tiny)
            qvT = p1.tile([P, 4, S], F32, tag="qvT")
            for co in range(4):
                ps = pp1x.tile([P, S], F32, tag="qv")
                for ci in range(4):
                    nc.tensor.matmul(
                        ps,
                        wv_wF[:, ci, co * P:(co + 1) * P],
                        query_t[:, ci, :],
                        start=(ci == 0), stop=(ci == 3),
                    )
                nc.vector.tensor_copy(out=qvT[:, co, :], in_=ps)

            # qb[s] = query @ wv_b; abias2 = abias + qb
            abias2 = p1.tile([S, 1], F32, tag="abias2")
            ps_qb = pp1x.tile([S, 1], F32, tag="qb")
            for ci in range(4):
                nc.tensor.matmul(
                    ps_qb,
                    query_t[:, ci, :],
                    wv_b[:, ci, :],
                    start=(ci == 0), stop=(ci == 3),
                )
            nc.vector.tensor_tensor(out=abias2, in0=ps_qb, in1=abias, op=OP.add)

            for n0, nsz in N_TILES:
                key_t = p1k.tile([P, 4, 512], F32, tag="key_t")
                nc.sync.dma_start(key_t[:, :, :nsz], key_v[:, :, n0:n0 + nsz])
                key_rt = p1k.tile([P, 4, 512], F32R, tag="key_rt")
                nc.sync.dma_start(key_rt[:, :, :nsz], key_rv[:, :, n0:n0 + nsz])

                # word-attention logits (fp32)
                ps_lg = pp1.tile([S, 512], F32, tag="lg")
                for ci in range(4):
                    nc.tensor.matmul(
                        ps_lg[:, :nsz],
                        qvT[:, ci, :],
                        key_t[:, ci, :nsz],
                        start=(ci == 0), stop=(ci == 3),
                    )
                # exp(logit - C1 + mask_bias + qb)
                exp_t = p1t.tile([S, 512], F32R, tag="exp1")
                nc.scalar.activation(
                    out=exp_t[:, :nsz], in_=ps_lg[:, :nsz],
                    func=AF.Exp, bias=abias2, scale=1.0,
                )
                # colsum over S, reciprocal, broadcast to 20 partitions
                ps_cs = pp1x.tile([1, 512], F32, tag="aux1")
                nc.tensor.matmul(
                    ps_cs[:, :nsz],
                    ones20, exp_t[:, :nsz],
                    start=True, stop=True,
                )
                rc1 = p1t.tile([1, 512], F32, tag="rc1")
                nc.vector.reciprocal(out=rc1[:, :nsz], in_=ps_cs[:, :nsz])
                ps_bc = pp1x.tile([S, 512], F32, tag="aux1")
                nc.tensor.matmul(
                    ps_bc[:, :nsz],
                    ones_1x20, rc1[:, :nsz],
                    start=True, stop=True,
                )
                rc1_bc = p1t.tile([S, 512], F32, tag="rc1bc")
                nc.scalar.activation(
                    out=rc1_bc[:, :nsz], in_=ps_bc[:, :nsz],
                    func=AF.Identity, bias=0.0, scale=1.0,
                )
                attn_t = p1t.tile([S, 512], F32R, tag="attn")
                nc.vector.tensor_tensor(
                    out=attn_t[:, :nsz], in0=exp_t[:, :nsz],
                    in1=rc1_bc[:, :nsz], op=OP.mult,
                )

                # wemb[c, n] = query.T @ attn
                for co in range(4):
                    ps_we = ppcv.tile([P, 512], F32, tag="conv")
                    nc.tensor.matmul(
                        ps_we[:, :nsz],
                        query[:, co * P:(co + 1) * P],
                        attn_t[:, :nsz],
                        start=True, stop=True,
                    )
                    nc.vector.tensor_copy(
                        out=wemb[:, co, n0:n0 + nsz], in_=ps_we[:, :nsz]
                    )

                # k_feat = fk_w @ key + fk_b
                for kc in range(2):
                    ps_kf = ppcv.tile([P, 512], F32, tag="conv")
                    for ci in range(4):
                        nc.tensor.matmul(
                            ps_kf[:, :nsz],
                            fk_wT[:, ci, kc * P:(kc + 1) * P],
                            key_rt[:, ci, :nsz],
                            start=(ci == 0), stop=(ci == 3),
                        )
                    nc.scalar.activation(
                        out=kf[:, kc, n0:n0 + nsz], in_=ps_kf[:, :nsz],
                        func=AF.Identity, bias=fk_b[:, kc, :], scale=1.0,
                    )

                # q_feat = fq_w @ wemb + fq_b
                for kc in range(2):
                    ps_qf = ppcv.tile([P, 512], F32, tag="conv")
                    for ci in range(4):
                        nc.tensor.matmul(
                            ps_qf[:, :nsz],
                            fq_wT[:, ci, kc * P:(kc + 1) * P],
                            wemb[:, ci, n0:n0 + nsz],
                            start=(ci == 0), stop=(ci == 3),
                        )
                    nc.scalar.activation(
                        out=qf[:, kc, n0:n0 + nsz], in_=ps_qf[:, :nsz],
                        func=AF.Identity, bias=fq_b[:, kc, :], scale=1.0,
                    )

                # v_T[m, c] = key_m.T @ fv_wT  (bias folded into ctx later)
                for mi in range(nsz // P):
                    m = n0 // P + mi
                    ps_vt = ppcv.tile([P, 512], F32, tag="conv")
                    for ci in range(4):
                        nc.tensor.matmul(
                            ps_vt,
                            key_rt[:, ci, mi * P:(mi + 1) * P],
                            fv_wT[:, ci, :],
                            start=(ci == 0), stop=(ci == 3),
                        )
                    nc.vector.tensor_copy(out=vt[:, m, :], in_=ps_vt)

        # ================= phase 2 =================
        with tc.tile_pool(name=f"p2exp{rep}", bufs=3) as p2e, \
             tc.tile_pool(name=f"p2sb{rep}", bufs=2) as p2sb, \
             tc.tile_pool(name=f"p2out{rep}", bufs=3) as p2o, \
             tc.tile_pool(name=f"psctx{rep}", bufs=1, space="PSUM") as ppc, \
             tc.tile_pool(name=f"psst{rep}", bufs=2, space="PSUM") as pps, \
             tc.tile_pool(name=f"psaux{rep}", bufs=1, space="PSUM") as ppa, \
             tc.tile_pool(name=f"psout{rep}", bufs=1, space="PSUM") as ppo:
            for n0, nsz in N_TILES:
                ps_ctx = [ppc.tile([P, 512], F32, tag=f"ctx{c}", name=f"ps_ctx{c}") for c in range(4)]
                ps_cs2 = ppa.tile([1, 512], F32, tag="aux2")

                for m in range(M_CHUNKS):
                    ps_s = pps.tile([P, 512], F32, tag="sT")
                    for kc in range(2):
                        nc.tensor.matmul(
                            ps_s[:, :nsz],
                            kf[:, kc, m * P:(m + 1) * P],
                            qf[:, kc, n0:n0 + nsz],
                            start=(kc == 0), stop=(kc == 1),
                        )
                    exp_t = p2e.tile([P, 512], F32R, tag="exp2")
                    nc.scalar.activation(
                        out=exp_t[:, :nsz], in_=ps_s[:, :nsz],
                        func=AF.Exp, bias=0.0, scale=SCALE2,
                    )
                    for c in range(4):
                        nc.tensor.matmul(
                            ps_ctx[c][:, :nsz],
                            vt[:, m, c * P:(c + 1) * P],
                            exp_t[:, :nsz],
                            start=(m == 0), stop=False,
                        )
                    nc.tensor.matmul(
                        ps_cs2[:, :nsz],
                        ones128, exp_t[:, :nsz],
                        start=(m == 0), stop=(m == M_CHUNKS - 1),
                    )

                # colsum -> sbuf, reciprocal, fold fv_b, broadcast recip
                cs_row = p2sb.tile([1, 512], F32R, tag="cs_row")
                nc.vector.tensor_copy(out=cs_row[:, :nsz], in_=ps_cs2[:, :nsz])
                rc2 = p2sb.tile([1, 512], F32, tag="rc2")
                nc.vector.reciprocal(out=rc2[:, :nsz], in_=cs_row[:, :nsz])
                # ctx_unnorm += fv_b (x) colsum   (K=1 matmul per c-chunk)
                for c in range(4):
                    nc.tensor.matmul(
                        ps_ctx[c][:, :nsz],
                        fv_b_row[:, c * P:(c + 1) * P],
                        cs_row[:, :nsz],
                        start=False, stop=True,
                    )
                ps_bc2 = ppa.tile([P, 512], F32, tag="aux2")
                nc.tensor.matmul(
                    ps_bc2[:, :nsz],
                    ones_1x128, rc2[:, :nsz],
                    start=True, stop=True,
                )
                rc2_bc = p2sb.tile([P, 512], F32, tag="rc2bc")
                nc.scalar.activation(
                    out=rc2_bc[:, :nsz], in_=ps_bc2[:, :nsz],
                    func=AF.Identity, bias=0.0, scale=1.0,
                )
                ctx_sb = [p2sb.tile([P, 512], F32R, tag=f"ctxsb{c}", name=f"ctx_sb{c}") for c in range(4)]
                for c in range(4):
                    nc.vector.tensor_tensor(
                        out=ctx_sb[c][:, :nsz], in0=ps_ctx[c][:, :nsz],
                        in1=rc2_bc[:, :nsz], op=OP.mult,
                    )

                # out = w1a @ ctx + w2 @ ctx + w1b @ wemb + bo
                for co in range(4):
                    ps_out = ppo.tile([P, 512], F32, tag="out")
                    first = True
                    for wT, rhs_fn in (
                        (w1aT, lambda ci: ctx_sb[ci][:, :nsz]),
                        (w2T, lambda ci: ctx_sb[ci][:, :nsz]),
                        (w1bT, lambda ci: wemb[:, ci, n0:n0 + nsz]),
                    ):
                        for ci in range(4):
                            nc.tensor.matmul(
                                ps_out[:, :nsz],
                                wT[:, ci, co * P:(co + 1) * P],
                                rhs_fn(ci),
                                start=first, stop=(wT is w1bT and ci == 3),
                            )
                            first = False
                    out_t = p2o.tile([P, 512], F32, tag="out_t")
                    nc.vector.tensor_tensor(
                        out=out_t[:, :nsz], in0=ps_out[:, :nsz],
                        in1=bo[:, co, :].to_broadcast((P, nsz)), op=OP.add,
                    )
                    nc.sync.dma_start(out_v[:, co, n0:n0 + nsz], out_t[:, :nsz])


_NC_CACHE = {}


def _build_program(reps=1):
    nc = bacc.Bacc("TRN2", target_bir_lowering=False, debug=False, num_devices=B)
    with tile.TileContext(nc) as tc:
        din, out_d = _declare_io(nc)
        for r in range(reps):
            _build_body(nc, tc, din, out_d, rep=r)
    nc.compile()
    return nc


def _get_program(reps=1):
    if reps not in _NC_CACHE:
        _NC_CACHE[reps] = _build_program(reps)
    return _NC_CACHE[reps]


def _prep_inputs(key, query, word_id, wv_w, wv_b, fk_w, fk_b, fq_w, fq_b,
                 fv_w, fv_b, w1_w, w1_b, w2_w, w2_b):
    f = lambda x: np.ascontiguousarray(np.asarray(x), dtype=np.float32)
    key = f(key).reshape(B, C, HW)
    query = f(query)
    word_id = np.asarray(word_id)
    shared = {
        "wv_wF": f(wv_w),
        "fk_wT": f(fk_w.T),
        "fq_wT": f(fq_w.T),
        "fv_wT": f(fv_w.T),
        "w1aT": f(w1_w[:, :C].T),
        "w1bT": f(w1_w[:, C:].T),
        "w2T": f(w2_w.T),
        "wv_b": f(wv_b).reshape(C, 1),
        "fk_b": f(fk_b).reshape(KC, 1),
        "fq_b": f(fq_b).reshape(KC, 1),
        "fv_b_row": f(fv_b).reshape(1, C),
        "bo": (f(w1_b) + f(w2_b)).reshape(C, 1),
        "ones_col": np.ones((P, 1), np.float32),
    }
    in_maps = []
    for b in range(B):
        abias = np.where(word_id[b] != 0, -C1, MASK_NEG).astype(np.float32)
        in_maps.append({
            "key": key[b],
            "key_r": key[b],
            "query": query[b],
            "query_t": np.ascontiguousarray(query[b].T),
            "abias": abias.reshape(S, 1),
            **shared,
        })
    return in_maps


def kernel(**inputs):
    from concourse.bass_utils import run_bass_kernel_spmd

    nc = _get_program()
    in_maps = _prep_inputs(**inputs)
    res = run_bass_kernel_spmd(nc, in_maps, core_ids=list(range(B)))
    out = np.stack([res.results[b]["out"].reshape(C, H, W) for b in range(B)])
    return out.astype(np.float32)


def run_traced(**inputs):
    """Like kernel() but with NTFF tracing; returns (out, BassKernelResults)."""
    from concourse.bass_utils import run_bass_kernel_spmd

    nc = _get_program()
    in_maps = _prep_inputs(**inputs)
    res = run_bass_kernel_spmd(nc, in_maps, core_ids=list(range(B)), trace=True)
    out = np.stack([res.results[b]["out"].reshape(C, H, W) for b in range(B)])
    return out.astype(np.float32), res

